# revision 16
# baseline (speedup 1.0000x reference)
"""AttentionBlock kernel for 8 TRN2 NeuronCores.

Problem: GroupNorm(32) -> QKV proj (4 heads, d_k=64) -> softmax attention
-> out proj -> residual, on x [4, 256, 64, 64] fp32.

Sharding: 8 cores = (batch b in 0..3) x (query-half in 0..1). Every core
computes GroupNorm + K/V for its full image (duplicated across the pair of
cores sharing a batch), Q/attention/output-projection/residual for its own
2048 query positions. Host-side gather is a pure concatenation.

Layout notes:
- Everything feature-major [C, N] on chip, the natural layout of x [C, H*W].
- Attention is computed transposed: S^T[j, i] = K^T-block matmuls, so the
  softmax denominator comes from a ones-column fused into the V matmul
  (M = 65) and P^T @ V -> O^T feeds the output projection directly.
- exp has no max-subtraction: logits for this problem are < 1 in magnitude.
- Bias algebra: the K bias cancels inside softmax, the V bias is folded
  into the output-projection bias on the host. Attention scale is folded
  into Wq/Wk, 1/sqrt(2) into Wout/bout.
- The kernel is ACT-bound (33.5M exp elements/core through the activation
  LUT). Everything else hides under it: head-pair 1 projections are
  emitted inside head-pair 0's attention, softmax normalization is
  deferred to the epilogue, S psum double-buffered, O accumulators
  drained by cheap copies.
- QK/PV matmuls run in bf16, projections in float32r (both full PE rate;
  the attention path is ~2% of the output magnitude, so the noise lands
  ~1e-4 relative on the final output).
"""

import math

import numpy as np

import concourse.bass as bass
import concourse.bacc as bacc
import concourse.tile as tile
from concourse import mybir
from concourse import bass_utils

F32 = mybir.dt.float32
F32R = mybir.dt.float32r
BF16 = mybir.dt.bfloat16

B = 4
C = 256
HW = 4096          # 64*64 spatial positions
NQ = HW // 2       # query positions owned by one core
N_HEADS = 4
D_K = 64
N_GROUPS = 32
EPS = 1e-5
SCALE = 1.0 / math.sqrt(math.sqrt(D_K))
INV_SQRT2 = 1.0 / math.sqrt(2.0)

CT = C // 128      # channel tiles (2)
JT = HW // 128     # key tiles (32)
ICH = NQ // 512    # query chunks of 512 (4)
ADD = mybir.AluOpType.add
MULT = mybir.AluOpType.mult


def _emit(nc, tc, t):
    """Emit the per-core program. `t` maps names -> dram APs."""
    import contextlib

    ctx = contextlib.ExitStack()
    with ctx:
        singles = ctx.enter_context(tc.tile_pool(name="singles", bufs=1))
        big = ctx.enter_context(tc.tile_pool(name="big", bufs=1))
        work = ctx.enter_context(tc.tile_pool(name="work", bufs=3))

        # ---- constants ----
        gb = singles.tile([128, 2 * CT], F32, tag="gb")  # gamma | beta per ctile
        for ct in range(CT):
            nc.sync.dma_start(out=gb[:, ct : ct + 1], in_=t["gamma"][ct * 128 : (ct + 1) * 128, :])
            nc.sync.dma_start(out=gb[:, CT + ct : CT + ct + 1], in_=t["beta"][ct * 128 : (ct + 1) * 128, :])
        wq = singles.tile([128, CT * 256], F32R, tag="wq")  # [c_part, ct*256 + feat]
        wk = singles.tile([128, CT * 256], F32R, tag="wk")
        wv = singles.tile([128, CT * 256], F32R, tag="wv")
        for ct in range(CT):
            nc.sync.dma_start(out=wq[:, ct * 256 : (ct + 1) * 256], in_=t["wq"][ct * 128 : (ct + 1) * 128, :])
            nc.sync.dma_start(out=wk[:, ct * 256 : (ct + 1) * 256], in_=t["wk"][ct * 128 : (ct + 1) * 128, :])
            nc.sync.dma_start(out=wv[:, ct * 256 : (ct + 1) * 256], in_=t["wv"][ct * 128 : (ct + 1) * 128, :])
        wo = singles.tile([128, 2 * 256], BF16, tag="wo")  # [dh_part, hp*256 + c]
        for hp in range(2):
            nc.sync.dma_start(out=wo[:, hp * 256 : (hp + 1) * 256], in_=t["wout"][hp * 128 : (hp + 1) * 128, :])
        bq_sb = singles.tile([128, 2], F32, tag="bq_sb")
        for ft in range(2):
            nc.sync.dma_start(out=bq_sb[:, ft : ft + 1], in_=t["bq"][ft * 128 : (ft + 1) * 128, :])
        bo = singles.tile([128, CT], F32, tag="bo")
        for ct in range(CT):
            nc.sync.dma_start(out=bo[:, ct : ct + 1], in_=t["bout"][ct * 128 : (ct + 1) * 128, :])
        ones1 = singles.tile([1, 64], F32, tag="ones1")
        nc.vector.memset(ones1, 1.0)
        gmap = singles.tile([128, 16], F32, tag="gmap")
        nc.sync.dma_start(out=gmap, in_=t["gmap"])
        gmapT = singles.tile([16, 128], F32, tag="gmapT")
        nc.sync.dma_start(out=gmapT, in_=t["gmapT"])

        # ---- persistent big tensors ----
        xr = []
        for ct in range(CT):
            xrt = big.tile([128, NQ], F32, tag=f"xr{ct}", name=f"xr{ct}")
            nc.sync.dma_start(out=xrt, in_=t["x_res"][ct * 128 : (ct + 1) * 128, :])
            xr.append(xrt)
        QT = [big.tile([128, NQ], BF16, tag=f"QT{ft}", name=f"QT{ft}") for ft in range(2)]
        KT = [big.tile([128, HW], BF16, tag=f"KT{ft}", name=f"KT{ft}") for ft in range(2)]
        # Vaug[ft] [token, jt, 130]: 0:64 V head even | 64 ones | 65:129 V
        # head odd | 129 ones
        Vaug = [big.tile([128, JT, 130], BF16, tag=f"Vaug{ft}", name=f"Vaug{ft}") for ft in range(2)]
        # OT holds UNNORMALIZED O^T rows during the attention loops; the
        # softmax denominators land in zall and normalization happens in
        # the epilogue (off the ACT critical path).
        OT = [big.tile([128, NQ], BF16, tag=f"OT{ft}", name=f"OT{ft}") for ft in range(2)]
        zall = big.tile([1, 2 * ICH * 2 * 512], F32, tag="zall")

        apsum = ctx.enter_context(tc.tile_pool(name="apsum", bufs=1, space="PSUM"))

        def emit_proj(ft, xs, hq):
            """Q/K projections (feature-major) + V (token-major into Vaug)."""
            for ch in range(ICH):
                q_ps = apsum.tile([128, 512], F32, tag="proj", name="q_ps", bufs=2)
                for ct in range(CT):
                    nc.tensor.matmul(
                        out=q_ps,
                        lhsT=wq[:, ct * 256 + ft * 128 : ct * 256 + (ft + 1) * 128],
                        rhs=hq[ct][:, ch * 512 : (ch + 1) * 512],
                        start=(ct == 0), stop=(ct == CT - 1),
                    )
                nc.vector.tensor_scalar_add(
                    out=QT[ft][:, ch * 512 : (ch + 1) * 512], in0=q_ps, scalar1=bq_sb[:, ft : ft + 1]
                )
            for ch in range(HW // 512):
                k_ps = apsum.tile([128, 512], F32, tag="proj", name="k_ps", bufs=2)
                for ct in range(CT):
                    nc.tensor.matmul(
                        out=k_ps,
                        lhsT=wk[:, ct * 256 + ft * 128 : ct * 256 + (ft + 1) * 128],
                        rhs=xs[ct][:, ch * 512 : (ch + 1) * 512],
                        start=(ct == 0), stop=(ct == CT - 1),
                    )
                nc.vector.tensor_copy(out=KT[ft][:, ch * 512 : (ch + 1) * 512], in_=k_ps)
            nc.vector.memset(Vaug[ft][:, :, 64:65], 1.0)
            nc.vector.memset(Vaug[ft][:, :, 129:130], 1.0)
            for nb in range(JT):
                v_ps = apsum.tile([128, 128], F32, tag="proj", name="v_ps", bufs=2)
                for ct in range(CT):
                    nc.tensor.matmul(
                        out=v_ps,
                        lhsT=xs[ct][:, nb * 128 : (nb + 1) * 128],
                        rhs=wv[:, ct * 256 + ft * 128 : ct * 256 + (ft + 1) * 128],
                        start=(ct == 0), stop=(ct == CT - 1),
                    )
                dst = bass.AP(
                    tensor=Vaug[ft].tensor, offset=Vaug[ft].offset + nb * 130,
                    ap=[Vaug[ft].ap[0], [65, 2], [1, 64]],
                )
                nc.vector.tensor_copy(
                    out=dst, in_=v_ps.rearrange("p (h d) -> p h d", d=64)
                )

        def emit_attn_chunk(ft, ic):
            """One (head-pair, 512-query) attention chunk; leaves O
            unnormalized in OT and the denominators in zall."""
            o_ps = [
                apsum.tile([65, 512], F32, tag=f"O{h}", name=f"O{h}", bufs=1)
                for h in range(2)
            ]
            for jt in range(JT):
                s_ps = apsum.tile([128, 1024], F32, tag="S", name="s_ps", bufs=2)
                for h in range(2):
                    nc.tensor.matmul(
                        out=s_ps[:, h * 512 : (h + 1) * 512],
                        lhsT=KT[ft][h * 64 : (h + 1) * 64, jt * 128 : (jt + 1) * 128],
                        rhs=QT[ft][h * 64 : (h + 1) * 64, ic * 512 : (ic + 1) * 512],
                        start=True, stop=True,
                    )
                pt = work.tile([128, 1024], BF16, tag="PT", name="pt")
                nc.scalar.activation(out=pt, in_=s_ps, func=mybir.ActivationFunctionType.Exp)
                for h in range(2):
                    nc.tensor.matmul(
                        out=o_ps[h],
                        lhsT=Vaug[ft][:, jt, h * 65 : (h + 1) * 65],
                        rhs=pt[:, h * 512 : (h + 1) * 512],
                        start=(jt == 0), stop=(jt == JT - 1),
                    )
            for h in range(2):
                nc.vector.tensor_copy(
                    out=OT[ft][h * 64 : (h + 1) * 64, ic * 512 : (ic + 1) * 512],
                    in_=o_ps[h][0:64, :],
                )
                zi = ((ft * ICH + ic) * 2 + h) * 512
                nc.vector.tensor_copy(out=zall[:, zi : zi + 512], in_=o_ps[h][64:65, :])

        # ================= prologue: GroupNorm + head-pair 0 QKV =========
        with tc.tile_pool(name="pxs", bufs=1) as pxs:
            xs = []
            for ct in range(CT):
                xt = pxs.tile([128, HW], F32R, tag=f"xs{ct}", name=f"xs{ct}")
                nc.sync.dma_start(out=xt, in_=t["x_full"][ct * 128 : (ct + 1) * 128, :])
                xs.append(xt)

            # per-channel mean / E[x^2] via bn_stats
            mv2 = []
            for ct in range(CT):
                st = work.tile([128, 8, 6], F32, tag="bnst", name="bnst")
                for k in range(8):
                    nc.vector.bn_stats(out=st[:, k, :], in_=xs[ct][:, k * 512 : (k + 1) * 512])
                mv = work.tile([128, 2], F32, tag="bnmv", name="bnmv")
                nc.vector.bn_aggr(out=mv, in_=st)
                m = work.tile([128, 2], F32, tag="mv2", name="mv2")
                nc.vector.tensor_copy(out=m[:, 0:1], in_=mv[:, 0:1])
                nc.vector.scalar_tensor_tensor(  # E[x^2] = var + mean^2
                    out=m[:, 1:2], in0=mv[:, 0:1], scalar=mv[:, 0:1], in1=mv[:, 1:2],
                    op0=MULT, op1=ADD,
                )
                mv2.append(m)

            # combine 8-channel groups via tiny G-matrix matmuls
            gsb = work.tile([16, 2, CT], F32, tag="gsb")
            for ct in range(CT):
                gs_ps = apsum.tile([16, 2], F32, tag="proj", name="gs_ps", bufs=2)
                nc.tensor.matmul(out=gs_ps, lhsT=gmap, rhs=mv2[ct], start=True, stop=True)
                nc.vector.tensor_copy(out=gsb[:, :, ct], in_=gs_ps)
            gmn = work.tile([16, CT], F32, tag="gmn")    # group mean
            nc.vector.tensor_scalar_mul(out=gmn, in0=gsb[:, 0, :], scalar1=1.0 / 8.0)
            gvar = work.tile([16, CT], F32, tag="gvar")  # group var
            nc.vector.tensor_scalar_mul(out=gvar, in0=gsb[:, 1, :], scalar1=1.0 / 8.0)
            gmsq = work.tile([16, CT], F32, tag="gmsq")
            nc.vector.tensor_mul(out=gmsq, in0=gmn, in1=gmn)
            nc.vector.tensor_sub(out=gvar, in0=gvar, in1=gmsq)
            eps_t = work.tile([16, 1], F32, tag="eps")
            nc.vector.memset(eps_t, EPS)
            nc.scalar.activation(
                out=gvar, in_=gvar, func=mybir.ActivationFunctionType.Sqrt,
                bias=eps_t, scale=1.0,
            )
            grs = work.tile([16, CT], F32, tag="grs")    # group rstd
            nc.vector.reciprocal(out=grs, in_=gvar)

            # broadcast group (mean, rstd) back to channel partitions
            coeff = []  # [128, 2]: a = gamma*rstd, b2 = beta - mean*a
            for ct in range(CT):
                mrs = work.tile([16, 2], F32, tag="mrs", name="mrs")
                nc.vector.tensor_copy(out=mrs[:, 0:1], in_=gmn[:, ct : ct + 1])
                nc.vector.tensor_copy(out=mrs[:, 1:2], in_=grs[:, ct : ct + 1])
                ch_ps = apsum.tile([128, 2], F32, tag="proj", name="ch_ps", bufs=2)
                nc.tensor.matmul(out=ch_ps, lhsT=gmapT, rhs=mrs, start=True, stop=True)
                mr = work.tile([128, 2], F32, tag="mr", name="mr")
                nc.vector.tensor_copy(out=mr, in_=ch_ps)
                cf = work.tile([128, 2], F32, tag=f"coeff{ct}", name=f"coeff{ct}")
                nc.vector.tensor_mul(out=cf[:, 0:1], in0=gb[:, ct : ct + 1], in1=mr[:, 1:2])
                na = work.tile([128, 1], F32, tag="na", name="na")
                nc.vector.tensor_scalar_mul(out=na, in0=cf[:, 0:1], scalar1=-1.0)
                nc.vector.scalar_tensor_tensor(
                    out=cf[:, 1:2], in0=mr[:, 0:1], scalar=na, in1=gb[:, CT + ct : CT + ct + 1],
                    op0=MULT, op1=ADD,
                )
                coeff.append(cf)

            # normalize: hid in place over xs; hid_q from x_res
            hq = []
            for ct in range(CT):
                nc.vector.tensor_scalar(
                    out=xs[ct], in0=xs[ct], scalar1=coeff[ct][:, 0:1], scalar2=coeff[ct][:, 1:2],
                    op0=MULT, op1=ADD,
                )
                h = pxs.tile([128, NQ], F32R, tag=f"hq{ct}", name=f"hq{ct}")
                nc.vector.tensor_scalar(
                    out=h, in0=xr[ct], scalar1=coeff[ct][:, 0:1], scalar2=coeff[ct][:, 1:2],
                    op0=MULT, op1=ADD,
                )
                hq.append(h)

            emit_proj(0, xs, hq)

            # ====== head-pair 0 attention; pair 1 projections overlap ====
            emit_attn_chunk(0, 0)
            emit_proj(1, xs, hq)  # PE fills ACT shadow of chunk (0, 0)
            for ic in range(1, ICH):
                emit_attn_chunk(0, ic)

        # ================= head-pair 1 attention =================
        for ic in range(ICH):
            emit_attn_chunk(1, ic)

        # ================= epilogue =================
        # softmax normalization of OT (reciprocal of denominators,
        # partition-broadcast via rank-1 matmul), then output projection
        # + residual.
        for ft in range(2):
            for ic in range(ICH):
                for h in range(2):
                    zi = ((ft * ICH + ic) * 2 + h) * 512
                    rec = work.tile([1, 512], F32, tag="rec", name="rec")
                    nc.vector.reciprocal_approx_fast(out=rec, in_=zall[:, zi : zi + 512])
                    recb = apsum.tile([64, 512], F32, tag="proj", name="recb", bufs=2)
                    nc.tensor.matmul(out=recb, lhsT=ones1, rhs=rec, start=True, stop=True)
                    sl = OT[ft][h * 64 : (h + 1) * 64, ic * 512 : (ic + 1) * 512]
                    nc.vector.scalar_tensor_tensor(
                        out=sl, in0=recb, scalar=1.0, in1=sl, op0=MULT, op1=MULT,
                    )
        for ct in range(CT):
            for ch in range(ICH):
                y_ps = apsum.tile([128, 512], F32, tag="proj", name="y_ps", bufs=2)
                for hp in range(2):
                    nc.tensor.matmul(
                        out=y_ps,
                        lhsT=wo[:, hp * 256 + ct * 128 : hp * 256 + (ct + 1) * 128],
                        rhs=OT[hp][:, ch * 512 : (ch + 1) * 512],
                        start=(hp == 0), stop=(hp == 1),
                    )
                yb = work.tile([128, 512], F32, tag="yb", name="yb")
                nc.vector.tensor_scalar_add(out=yb, in0=y_ps, scalar1=bo[:, ct : ct + 1])
                osb = work.tile([128, 512], F32, tag="osb", name="osb")
                nc.vector.scalar_tensor_tensor(
                    out=osb, in0=xr[ct][:, ch * 512 : (ch + 1) * 512], scalar=INV_SQRT2, in1=yb,
                    op0=MULT, op1=ADD,
                )
                nc.sync.dma_start(
                    out=t["out"][ct * 128 : (ct + 1) * 128, ch * 512 : (ch + 1) * 512], in_=osb
                )


def build_nc():
    nc = bacc.Bacc("TRN2", target_bir_lowering=False, debug=False)
    t = {}
    def inp(name, shape, dt=F32):
        t[name] = nc.dram_tensor(name, shape, dt, kind="ExternalInput").ap()
    inp("x_full", [C, HW], F32R)
    inp("x_res", [C, NQ])
    inp("wq", [C, 256], F32R)
    inp("wk", [C, 256], F32R)
    inp("wv", [C, 256], F32R)
    inp("bq", [C, 1])
    inp("wout", [256, C], BF16)
    inp("bout", [C, 1])
    inp("gamma", [C, 1])
    inp("beta", [C, 1])
    inp("gmap", [128, 16])
    inp("gmapT", [16, 128])
    t["out"] = nc.dram_tensor("out", [C, NQ], F32, kind="ExternalOutput").ap()
    with tile.TileContext(nc) as tc:
        _emit(nc, tc, t)
    nc.compile()
    return nc


def host_inputs(x, gamma, beta, W_qkv, b_qkv, W_out, b_out):
    """Shared (weights) and per-core (x slices) input maps."""
    import ml_dtypes

    x = np.ascontiguousarray(np.asarray(x, dtype=np.float32))
    gamma = np.asarray(gamma, dtype=np.float32)
    beta = np.asarray(beta, dtype=np.float32)
    W_qkv = np.asarray(W_qkv, dtype=np.float32)
    b_qkv = np.asarray(b_qkv, dtype=np.float32)
    W_out = np.asarray(W_out, dtype=np.float32)
    b_out = np.asarray(b_out, dtype=np.float32)

    cols = lambda off: np.concatenate(
        [W_qkv[:, h * 192 + off : h * 192 + off + 64] for h in range(N_HEADS)], axis=1
    )
    bcols = lambda off: np.concatenate(
        [b_qkv[h * 192 + off : h * 192 + off + 64] for h in range(N_HEADS)]
    )
    bv = bcols(128)
    bout_eff = (b_out + bv @ W_out) * INV_SQRT2
    shared = {
        "wq": np.ascontiguousarray(cols(0) * SCALE),
        "wk": np.ascontiguousarray(cols(64) * SCALE),
        "wv": np.ascontiguousarray(cols(128)),
        "bq": (bcols(0) * SCALE).reshape(C, 1),
        "wout": np.ascontiguousarray((W_out * INV_SQRT2).astype(ml_dtypes.bfloat16)),
        "bout": bout_eff.reshape(C, 1).astype(np.float32),
        "gamma": gamma.reshape(C, 1),
        "beta": beta.reshape(C, 1),
        "gmap": np.ascontiguousarray(np.kron(np.eye(16, dtype=np.float32), np.ones((8, 1), dtype=np.float32))),
        "gmapT": np.ascontiguousarray(np.kron(np.eye(16, dtype=np.float32), np.ones((1, 8), dtype=np.float32))),
    }
    in_maps = []
    for core in range(8):
        b, half = divmod(core, 2)
        xf = x[b].reshape(C, HW)
        m = dict(shared)
        m["x_full"] = xf
        m["x_res"] = np.ascontiguousarray(xf[:, half * NQ : (half + 1) * NQ])
        in_maps.append(m)
    return in_maps


def assemble(results):
    out = np.empty((B, C, HW), dtype=np.float32)
    for core in range(8):
        b, half = divmod(core, 2)
        out[b][:, half * NQ : (half + 1) * NQ] = results[core]["out"]
    return out.reshape(B, C, 64, 64)


_NC = None


def kernel(x, gamma, beta, W_qkv, b_qkv, W_out, b_out):
    global _NC
    if _NC is None:
        _NC = build_nc()
    in_maps = host_inputs(x, gamma, beta, W_qkv, b_qkv, W_out, b_out)
    res = bass_utils.run_bass_kernel_spmd(_NC, in_maps, core_ids=list(range(8)))
    return assemble(res.results)


# revision 19
# speedup vs baseline: 1.0243x; 1.0243x over previous
"""AttentionBlock kernel for 8 TRN2 NeuronCores.

Problem: GroupNorm(32) -> QKV proj (4 heads, d_k=64) -> softmax attention
-> out proj -> residual, on x [4, 256, 64, 64] fp32.

Sharding: 8 cores = (batch b in 0..3) x (query-half in 0..1). Every core
computes GroupNorm + K/V for its full image (duplicated across the pair of
cores sharing a batch), Q/attention/output-projection/residual for its own
2048 query positions. Host-side gather is a pure concatenation.

Layout notes:
- Everything feature-major [C, N] on chip, the natural layout of x [C, H*W].
- Attention is computed transposed: S^T[j, i] = K^T-block matmuls, so the
  softmax denominator comes from a ones-column fused into the V matmul
  (M = 65) and P^T @ V -> O^T feeds the output projection directly.
- exp has no max-subtraction: logits for this problem are < 1 in magnitude.
- Bias algebra: the K bias cancels inside softmax, the V bias is folded
  into the output-projection bias on the host. Attention scale is folded
  into Wq/Wk, 1/sqrt(2) into Wout/bout.
- The kernel is ACT-bound (33.5M exp elements/core through the activation
  LUT). Everything else hides under it: head-pair 1 projections are
  emitted inside head-pair 0's attention, softmax normalization is
  deferred to the epilogue, S psum double-buffered, O accumulators
  drained by cheap copies.
- QK/PV matmuls run in bf16, projections in float32r (both full PE rate;
  the attention path is ~2% of the output magnitude, so the noise lands
  ~1e-4 relative on the final output).
"""

import math

import numpy as np

import concourse.bass as bass
import concourse.bacc as bacc
import concourse.tile as tile
from concourse import mybir
from concourse import bass_utils

F32 = mybir.dt.float32
F32R = mybir.dt.float32r
BF16 = mybir.dt.bfloat16

B = 4
C = 256
HW = 4096          # 64*64 spatial positions
NQ = HW // 2       # query positions owned by one core
N_HEADS = 4
D_K = 64
N_GROUPS = 32
EPS = 1e-5
SCALE = 1.0 / math.sqrt(math.sqrt(D_K))
INV_SQRT2 = 1.0 / math.sqrt(2.0)

CT = C // 128      # channel tiles (2)
JT = HW // 128     # key tiles (32)
ICH = NQ // 512    # query chunks of 512 (4)
ADD = mybir.AluOpType.add
MULT = mybir.AluOpType.mult


def _emit(nc, tc, t):
    """Emit the per-core program. `t` maps names -> dram APs."""
    import contextlib

    ctx = contextlib.ExitStack()
    with ctx:
        singles = ctx.enter_context(tc.tile_pool(name="singles", bufs=1))
        big = ctx.enter_context(tc.tile_pool(name="big", bufs=1))
        work = ctx.enter_context(tc.tile_pool(name="work", bufs=3))

        # ---- constants ----
        gb = singles.tile([128, 2 * CT], F32, tag="gb")  # gamma | beta per ctile
        for ct in range(CT):
            nc.sync.dma_start(out=gb[:, ct : ct + 1], in_=t["gamma"][ct * 128 : (ct + 1) * 128, :])
            nc.sync.dma_start(out=gb[:, CT + ct : CT + ct + 1], in_=t["beta"][ct * 128 : (ct + 1) * 128, :])
        wq = singles.tile([128, CT * 256], F32R, tag="wq")  # [c_part, ct*256 + feat]
        wk = singles.tile([128, CT * 256], F32R, tag="wk")
        wv = singles.tile([128, CT * 256], F32R, tag="wv")
        for ct in range(CT):
            nc.sync.dma_start(out=wq[:, ct * 256 : (ct + 1) * 256], in_=t["wq"][ct * 128 : (ct + 1) * 128, :])
            nc.sync.dma_start(out=wk[:, ct * 256 : (ct + 1) * 256], in_=t["wk"][ct * 128 : (ct + 1) * 128, :])
            nc.sync.dma_start(out=wv[:, ct * 256 : (ct + 1) * 256], in_=t["wv"][ct * 128 : (ct + 1) * 128, :])
        wo = singles.tile([128, 2 * 256], BF16, tag="wo")  # [dh_part, hp*256 + c]
        for hp in range(2):
            nc.sync.dma_start(out=wo[:, hp * 256 : (hp + 1) * 256], in_=t["wout"][hp * 128 : (hp + 1) * 128, :])
        bq_sb = singles.tile([128, 2], F32, tag="bq_sb")
        for ft in range(2):
            nc.sync.dma_start(out=bq_sb[:, ft : ft + 1], in_=t["bq"][ft * 128 : (ft + 1) * 128, :])
        bo = singles.tile([128, CT], F32, tag="bo")
        for ct in range(CT):
            nc.sync.dma_start(out=bo[:, ct : ct + 1], in_=t["bout"][ct * 128 : (ct + 1) * 128, :])
        ones1 = singles.tile([1, 64], F32, tag="ones1")
        nc.vector.memset(ones1, 1.0)
        gmap = singles.tile([128, 16], F32, tag="gmap")
        nc.sync.dma_start(out=gmap, in_=t["gmap"])
        gmapT = singles.tile([16, 128], F32, tag="gmapT")
        nc.sync.dma_start(out=gmapT, in_=t["gmapT"])

        # ---- persistent big tensors ----
        xr = []
        for ct in range(CT):
            xrt = big.tile([128, NQ], F32, tag=f"xr{ct}", name=f"xr{ct}")
            nc.sync.dma_start(out=xrt, in_=t["x_res"][ct * 128 : (ct + 1) * 128, :])
            xr.append(xrt)
        QT = [big.tile([128, NQ], BF16, tag=f"QT{ft}", name=f"QT{ft}") for ft in range(2)]
        KT = [big.tile([128, HW], BF16, tag=f"KT{ft}", name=f"KT{ft}") for ft in range(2)]
        # Vaug[ft] [token, jt, 130]: 0:64 V head even | 64 ones | 65:129 V
        # head odd | 129 ones
        Vaug = [big.tile([128, JT, 130], BF16, tag=f"Vaug{ft}", name=f"Vaug{ft}") for ft in range(2)]
        # OT holds UNNORMALIZED O^T rows during the attention loops; the
        # softmax denominators land in zall and normalization happens in
        # the epilogue (off the ACT critical path).
        OT = [big.tile([128, NQ], BF16, tag=f"OT{ft}", name=f"OT{ft}") for ft in range(2)]
        zall = big.tile([1, 2 * ICH * 2 * 512], F32, tag="zall")

        apsum = ctx.enter_context(tc.tile_pool(name="apsum", bufs=1, space="PSUM"))

        def emit_q_chunk(ft, hq, ch):
            q_ps = apsum.tile([128, 512], F32, tag="proj", name="q_ps", bufs=2)
            for ct in range(CT):
                nc.tensor.matmul(
                    out=q_ps,
                    lhsT=wq[:, ct * 256 + ft * 128 : ct * 256 + (ft + 1) * 128],
                    rhs=hq[ct][:, ch * 512 : (ch + 1) * 512],
                    start=(ct == 0), stop=(ct == CT - 1),
                )
            nc.vector.tensor_scalar_add(
                out=QT[ft][:, ch * 512 : (ch + 1) * 512], in0=q_ps, scalar1=bq_sb[:, ft : ft + 1]
            )

        def emit_k_chunk(ft, xs, ch):
            k_ps = apsum.tile([128, 512], F32, tag="proj", name="k_ps", bufs=2)
            for ct in range(CT):
                nc.tensor.matmul(
                    out=k_ps,
                    lhsT=wk[:, ct * 256 + ft * 128 : ct * 256 + (ft + 1) * 128],
                    rhs=xs[ct][:, ch * 512 : (ch + 1) * 512],
                    start=(ct == 0), stop=(ct == CT - 1),
                )
            nc.vector.tensor_copy(out=KT[ft][:, ch * 512 : (ch + 1) * 512], in_=k_ps)

        def emit_v_block(ft, xs, nb):
            v_ps = apsum.tile([128, 128], F32, tag="proj", name="v_ps", bufs=2)
            for ct in range(CT):
                nc.tensor.matmul(
                    out=v_ps,
                    lhsT=xs[ct][:, nb * 128 : (nb + 1) * 128],
                    rhs=wv[:, ct * 256 + ft * 128 : ct * 256 + (ft + 1) * 128],
                    start=(ct == 0), stop=(ct == CT - 1),
                )
            dst = bass.AP(
                tensor=Vaug[ft].tensor, offset=Vaug[ft].offset + nb * 130,
                ap=[Vaug[ft].ap[0], [65, 2], [1, 64]],
            )
            nc.vector.tensor_copy(out=dst, in_=v_ps.rearrange("p (h d) -> p h d", d=64))

        def emit_attn_chunk(ft, ic, extras=()):
            """One (head-pair, 512-query) attention chunk; leaves O
            unnormalized in OT and the denominators in zall. `extras`
            are emitters sprinkled one-per-j-iteration so independent PE
            work (the other head pair's projections) fills ACT's shadow
            without forming a long program-order block."""
            extras = list(extras)
            o_ps = [
                apsum.tile([65, 512], F32, tag=f"O{h}", name=f"O{h}", bufs=1)
                for h in range(2)
            ]
            for jt in range(JT):
                s_ps = apsum.tile([128, 1024], F32, tag="S", name="s_ps", bufs=2)
                for h in range(2):
                    nc.tensor.matmul(
                        out=s_ps[:, h * 512 : (h + 1) * 512],
                        lhsT=KT[ft][h * 64 : (h + 1) * 64, jt * 128 : (jt + 1) * 128],
                        rhs=QT[ft][h * 64 : (h + 1) * 64, ic * 512 : (ic + 1) * 512],
                        start=True, stop=True,
                    )
                pt = work.tile([128, 1024], BF16, tag="PT", name="pt")
                nc.scalar.activation(out=pt, in_=s_ps, func=mybir.ActivationFunctionType.Exp)
                for h in range(2):
                    nc.tensor.matmul(
                        out=o_ps[h],
                        lhsT=Vaug[ft][:, jt, h * 65 : (h + 1) * 65],
                        rhs=pt[:, h * 512 : (h + 1) * 512],
                        start=(jt == 0), stop=(jt == JT - 1),
                    )
                if jt < len(extras):
                    extras[jt]()
            for h in range(2):
                nc.vector.tensor_copy(
                    out=OT[ft][h * 64 : (h + 1) * 64, ic * 512 : (ic + 1) * 512],
                    in_=o_ps[h][0:64, :],
                )
                zi = ((ft * ICH + ic) * 2 + h) * 512
                nc.vector.tensor_copy(out=zall[:, zi : zi + 512], in_=o_ps[h][64:65, :])

        def emit_normalize(ft, ic):
            """Divide OT rows of (ft, ic) by the softmax denominators."""
            for h in range(2):
                zi = ((ft * ICH + ic) * 2 + h) * 512
                rec = work.tile([1, 512], F32, tag="rec", name="rec")
                nc.vector.reciprocal_approx_fast(out=rec, in_=zall[:, zi : zi + 512])
                recb = apsum.tile([64, 512], F32, tag="proj", name="recb", bufs=2)
                nc.tensor.matmul(out=recb, lhsT=ones1, rhs=rec, start=True, stop=True)
                sl = OT[ft][h * 64 : (h + 1) * 64, ic * 512 : (ic + 1) * 512]
                nc.vector.scalar_tensor_tensor(
                    out=sl, in0=recb, scalar=1.0, in1=sl, op0=MULT, op1=MULT,
                )

        def emit_y_pass1(yacc, ch):
            """Head-pair 0 half of the output projection, into yacc."""
            for ct in range(CT):
                y_ps = apsum.tile([128, 512], F32, tag="proj", name="y_ps", bufs=2)
                nc.tensor.matmul(
                    out=y_ps,
                    lhsT=wo[:, ct * 128 : (ct + 1) * 128],
                    rhs=OT[0][:, ch * 512 : (ch + 1) * 512],
                    start=True, stop=True,
                )
                nc.vector.tensor_copy(out=yacc[ct][:, ch * 512 : (ch + 1) * 512], in_=y_ps)

        def emit_y_pass2(yacc, ch):
            """Head-pair 1 half + bias + yacc + residual, DMA out."""
            for ct in range(CT):
                y_ps = apsum.tile([128, 512], F32, tag="proj", name="y_ps", bufs=2)
                nc.tensor.matmul(
                    out=y_ps,
                    lhsT=wo[:, 256 + ct * 128 : 256 + (ct + 1) * 128],
                    rhs=OT[1][:, ch * 512 : (ch + 1) * 512],
                    start=True, stop=True,
                )
                yb = work.tile([128, 512], F32, tag="yb", name="yb")
                nc.vector.scalar_tensor_tensor(
                    out=yb, in0=y_ps, scalar=bo[:, ct : ct + 1],
                    in1=yacc[ct][:, ch * 512 : (ch + 1) * 512], op0=ADD, op1=ADD,
                )
                osb = work.tile([128, 512], F32, tag="osb", name="osb")
                nc.vector.scalar_tensor_tensor(
                    out=osb, in0=xr[ct][:, ch * 512 : (ch + 1) * 512], scalar=INV_SQRT2, in1=yb,
                    op0=MULT, op1=ADD,
                )
                nc.sync.dma_start(
                    out=t["out"][ct * 128 : (ct + 1) * 128, ch * 512 : (ch + 1) * 512], in_=osb
                )

        # ================= prologue: GroupNorm + head-pair 0 QKV =========
        with tc.tile_pool(name="pxs", bufs=1) as pxs:
            # x loaded in 512-column chunks so bn_stats pipelines with DMA
            xs = []
            for ct in range(CT):
                xt = pxs.tile([128, HW], F32R, tag=f"xs{ct}", name=f"xs{ct}")
                xs.append(xt)

            # per-channel mean / E[x^2] via bn_stats
            mv2 = []
            for ct in range(CT):
                st = work.tile([128, 8, 6], F32, tag="bnst", name="bnst")
                for k in range(8):
                    nc.sync.dma_start(
                        out=xs[ct][:, k * 512 : (k + 1) * 512],
                        in_=t["x_full"][ct * 128 : (ct + 1) * 128, k * 512 : (k + 1) * 512],
                    )
                    nc.vector.bn_stats(out=st[:, k, :], in_=xs[ct][:, k * 512 : (k + 1) * 512])
                mv = work.tile([128, 2], F32, tag="bnmv", name="bnmv")
                nc.vector.bn_aggr(out=mv, in_=st)
                m = work.tile([128, 2], F32, tag="mv2", name="mv2")
                nc.vector.tensor_copy(out=m[:, 0:1], in_=mv[:, 0:1])
                nc.vector.scalar_tensor_tensor(  # E[x^2] = var + mean^2
                    out=m[:, 1:2], in0=mv[:, 0:1], scalar=mv[:, 0:1], in1=mv[:, 1:2],
                    op0=MULT, op1=ADD,
                )
                mv2.append(m)

            # combine 8-channel groups via tiny G-matrix matmuls
            gsb = work.tile([16, 2, CT], F32, tag="gsb")
            for ct in range(CT):
                gs_ps = apsum.tile([16, 2], F32, tag="proj", name="gs_ps", bufs=2)
                nc.tensor.matmul(out=gs_ps, lhsT=gmap, rhs=mv2[ct], start=True, stop=True)
                nc.vector.tensor_copy(out=gsb[:, :, ct], in_=gs_ps)
            gmn = work.tile([16, CT], F32, tag="gmn")    # group mean
            nc.vector.tensor_scalar_mul(out=gmn, in0=gsb[:, 0, :], scalar1=1.0 / 8.0)
            gvar = work.tile([16, CT], F32, tag="gvar")  # group var
            nc.vector.tensor_scalar_mul(out=gvar, in0=gsb[:, 1, :], scalar1=1.0 / 8.0)
            gmsq = work.tile([16, CT], F32, tag="gmsq")
            nc.vector.tensor_mul(out=gmsq, in0=gmn, in1=gmn)
            nc.vector.tensor_sub(out=gvar, in0=gvar, in1=gmsq)
            eps_t = work.tile([16, 1], F32, tag="eps")
            nc.vector.memset(eps_t, EPS)
            nc.scalar.activation(
                out=gvar, in_=gvar, func=mybir.ActivationFunctionType.Sqrt,
                bias=eps_t, scale=1.0,
            )
            grs = work.tile([16, CT], F32, tag="grs")    # group rstd
            nc.vector.reciprocal(out=grs, in_=gvar)

            # broadcast group (mean, rstd) back to channel partitions
            coeff = []  # [128, 2]: a = gamma*rstd, b2 = beta - mean*a
            for ct in range(CT):
                mrs = work.tile([16, 2], F32, tag="mrs", name="mrs")
                nc.vector.tensor_copy(out=mrs[:, 0:1], in_=gmn[:, ct : ct + 1])
                nc.vector.tensor_copy(out=mrs[:, 1:2], in_=grs[:, ct : ct + 1])
                ch_ps = apsum.tile([128, 2], F32, tag="proj", name="ch_ps", bufs=2)
                nc.tensor.matmul(out=ch_ps, lhsT=gmapT, rhs=mrs, start=True, stop=True)
                mr = work.tile([128, 2], F32, tag="mr", name="mr")
                nc.vector.tensor_copy(out=mr, in_=ch_ps)
                cf = work.tile([128, 2], F32, tag=f"coeff{ct}", name=f"coeff{ct}")
                nc.vector.tensor_mul(out=cf[:, 0:1], in0=gb[:, ct : ct + 1], in1=mr[:, 1:2])
                na = work.tile([128, 1], F32, tag="na", name="na")
                nc.vector.tensor_scalar_mul(out=na, in0=cf[:, 0:1], scalar1=-1.0)
                nc.vector.scalar_tensor_tensor(
                    out=cf[:, 1:2], in0=mr[:, 0:1], scalar=na, in1=gb[:, CT + ct : CT + ct + 1],
                    op0=MULT, op1=ADD,
                )
                coeff.append(cf)

            # normalize: hid and hid_q chunk-wise, immediately followed by
            # the head-pair-0 projections of each chunk so the first
            # attention matmuls are reachable early.
            hq = []
            for ct in range(CT):
                h = pxs.tile([128, NQ], F32R, tag=f"hq{ct}", name=f"hq{ct}")
                hq.append(h)
            for ch in range(ICH):
                for ct in range(CT):
                    nc.vector.tensor_scalar(
                        out=hq[ct][:, ch * 512 : (ch + 1) * 512],
                        in0=xr[ct][:, ch * 512 : (ch + 1) * 512],
                        scalar1=coeff[ct][:, 0:1], scalar2=coeff[ct][:, 1:2],
                        op0=MULT, op1=ADD,
                    )
            emit_q_chunk(0, hq, 0)
            nc.vector.memset(Vaug[0][:, :, 64:65], 1.0)
            nc.vector.memset(Vaug[0][:, :, 129:130], 1.0)
            for ch in range(HW // 512):
                for ct in range(CT):
                    nc.vector.tensor_scalar(
                        out=xs[ct][:, ch * 512 : (ch + 1) * 512],
                        in0=xs[ct][:, ch * 512 : (ch + 1) * 512],
                        scalar1=coeff[ct][:, 0:1], scalar2=coeff[ct][:, 1:2],
                        op0=MULT, op1=ADD,
                    )
                emit_k_chunk(0, xs, ch)
                for nb in range(4 * ch, 4 * ch + 4):
                    emit_v_block(0, xs, nb)
            for ch in range(1, ICH):
                emit_q_chunk(0, hq, ch)

            # ====== head-pair 0 attention; pair 1 projections are
            # interleaved into the first chunks' j-iterations ======
            extras = []
            extras.append(lambda: nc.vector.memset(Vaug[1][:, :, 64:65], 1.0))
            extras.append(lambda: nc.vector.memset(Vaug[1][:, :, 129:130], 1.0))
            for ch in range(ICH):
                extras.append(lambda ch=ch: emit_q_chunk(1, hq, ch))
            for ch in range(HW // 512):
                extras.append(lambda ch=ch: emit_k_chunk(1, xs, ch))
            for nb in range(0, JT, 2):
                extras.append(lambda nb=nb: (emit_v_block(1, xs, nb), emit_v_block(1, xs, nb + 1)))

            yacc = [
                big.tile([128, NQ], F32, tag=f"yacc{ct}", name=f"yacc{ct}") for ct in range(CT)
            ]
            for ic in range(ICH):
                emit_attn_chunk(0, ic, extras=extras[ic * 8 : (ic + 1) * 8])
                emit_normalize(0, ic)
                emit_y_pass1(yacc, ic)

        # ============ head-pair 1 attention + epilogue per chunk ==========
        for ic in range(ICH):
            emit_attn_chunk(1, ic)
            emit_normalize(1, ic)
            emit_y_pass2(yacc, ic)


def build_nc():
    nc = bacc.Bacc("TRN2", target_bir_lowering=False, debug=False)
    t = {}
    def inp(name, shape, dt=F32):
        t[name] = nc.dram_tensor(name, shape, dt, kind="ExternalInput").ap()
    inp("x_full", [C, HW], F32R)
    inp("x_res", [C, NQ])
    inp("wq", [C, 256], F32R)
    inp("wk", [C, 256], F32R)
    inp("wv", [C, 256], F32R)
    inp("bq", [C, 1])
    inp("wout", [256, C], BF16)
    inp("bout", [C, 1])
    inp("gamma", [C, 1])
    inp("beta", [C, 1])
    inp("gmap", [128, 16])
    inp("gmapT", [16, 128])
    t["out"] = nc.dram_tensor("out", [C, NQ], F32, kind="ExternalOutput").ap()
    with tile.TileContext(nc) as tc:
        _emit(nc, tc, t)
    nc.compile()
    return nc


def host_inputs(x, gamma, beta, W_qkv, b_qkv, W_out, b_out):
    """Shared (weights) and per-core (x slices) input maps."""
    import ml_dtypes

    x = np.ascontiguousarray(np.asarray(x, dtype=np.float32))
    gamma = np.asarray(gamma, dtype=np.float32)
    beta = np.asarray(beta, dtype=np.float32)
    W_qkv = np.asarray(W_qkv, dtype=np.float32)
    b_qkv = np.asarray(b_qkv, dtype=np.float32)
    W_out = np.asarray(W_out, dtype=np.float32)
    b_out = np.asarray(b_out, dtype=np.float32)

    cols = lambda off: np.concatenate(
        [W_qkv[:, h * 192 + off : h * 192 + off + 64] for h in range(N_HEADS)], axis=1
    )
    bcols = lambda off: np.concatenate(
        [b_qkv[h * 192 + off : h * 192 + off + 64] for h in range(N_HEADS)]
    )
    bv = bcols(128)
    bout_eff = (b_out + bv @ W_out) * INV_SQRT2
    shared = {
        "wq": np.ascontiguousarray(cols(0) * SCALE),
        "wk": np.ascontiguousarray(cols(64) * SCALE),
        "wv": np.ascontiguousarray(cols(128)),
        "bq": (bcols(0) * SCALE).reshape(C, 1),
        "wout": np.ascontiguousarray((W_out * INV_SQRT2).astype(ml_dtypes.bfloat16)),
        "bout": bout_eff.reshape(C, 1).astype(np.float32),
        "gamma": gamma.reshape(C, 1),
        "beta": beta.reshape(C, 1),
        "gmap": np.ascontiguousarray(np.kron(np.eye(16, dtype=np.float32), np.ones((8, 1), dtype=np.float32))),
        "gmapT": np.ascontiguousarray(np.kron(np.eye(16, dtype=np.float32), np.ones((1, 8), dtype=np.float32))),
    }
    in_maps = []
    for core in range(8):
        b, half = divmod(core, 2)
        xf = x[b].reshape(C, HW)
        m = dict(shared)
        m["x_full"] = xf
        m["x_res"] = np.ascontiguousarray(xf[:, half * NQ : (half + 1) * NQ])
        in_maps.append(m)
    return in_maps


def assemble(results):
    out = np.empty((B, C, HW), dtype=np.float32)
    for core in range(8):
        b, half = divmod(core, 2)
        out[b][:, half * NQ : (half + 1) * NQ] = results[core]["out"]
    return out.reshape(B, C, 64, 64)


_NC = None


def kernel(x, gamma, beta, W_qkv, b_qkv, W_out, b_out):
    global _NC
    if _NC is None:
        _NC = build_nc()
    in_maps = host_inputs(x, gamma, beta, W_qkv, b_qkv, W_out, b_out)
    res = bass_utils.run_bass_kernel_spmd(_NC, in_maps, core_ids=list(range(8)))
    return assemble(res.results)
